# revision 71
# baseline (speedup 1.0000x reference)
"""Bahdanau attention Trainium2 kernel.

Problem sizes (hardcoded): B=64, N=2048, H=D=A=512, 8 NeuronCores,
data-parallel over B (8 batches per core, replicated weights).

Math per batch b:
    proj_h = h @ W_h.T                   (host: 0.05% of FLOPs)
    proj_e = enc_mem[b] @ W_e.T          (PE, bf16 inputs, fp32 accum)
    energy = tanh(proj_h + proj_e)       (ACT, bias fused)
    scores = energy @ v                  (PE, col-group packed 4x)
    scores += (mask-1)*1e9               (DVE, fused into psum->sbuf copy)
    alpha  = softmax(scores)             (constant-shift exp: bias=-sum|v|)
    context = (exp @ enc_mem[b]) / den   (PE col-group packed 4x)

Device layouts (per core, host-marshalled):
    encT  [8,128,4,2048] bf16 : encT[b,p,c,n] = enc[b,n,c*128+p]   (d-major)
    encN  [8,128,16,512] bf16 : encN[b,p,t,d] = enc[b,t*128+p,d]   (n-major)
    wT    [128,4,512]    bf16 : wT[p,c,a]     = W_e[a,c*128+p]
    vT    [128,4]        bf16 : vT[p,j]       = v[j*128+p]
    phT   [128,4,8]      f32  : phT[p,c,b]    = proj_h[b,c*128+p]
    mneg  [8,128,512]    bf16 : (mask-1)*1e9, rows spread to {0,32,64,96}
    nbias [4,1]          f32  : -sum(|v|) (constant softmax shift)
    ident [128,128]      bf16 : identity for PE transpose
Outputs: alpha [8,2048] f32, context [8,512] f32.
"""

import numpy as np
import ml_dtypes

B, N, D, A = 64, 2048, 512, 512
NCORES = 8
BL = B // NCORES          # 8 batches per core
GB = 4                    # batches per softmax group
NGROUPS = BL // GB        # 2 groups
BF16 = ml_dtypes.bfloat16

_PROG = None


def _build():
    import concourse.bass as bass
    import concourse.mybir as mybir
    import concourse.tile as tile
    from concourse import bacc

    dt = mybir.dt
    AF = mybir.ActivationFunctionType
    AX = mybir.AxisListType

    nc = bacc.Bacc("TRN2", target_bir_lowering=False, debug=False)

    encT = nc.dram_tensor("encT", [BL, 128, 4, N], dt.bfloat16, kind="ExternalInput")
    encN = nc.dram_tensor("encN", [BL, 128, 16, D], dt.bfloat16, kind="ExternalInput")
    wT = nc.dram_tensor("wT", [128, 4, A], dt.bfloat16, kind="ExternalInput")
    vT = nc.dram_tensor("vT", [128, 4, 32], dt.bfloat16, kind="ExternalInput")
    phT = nc.dram_tensor("phT", [128, 4, BL], dt.float32, kind="ExternalInput")
    mneg = nc.dram_tensor("mneg", [BL, 128, 512], dt.bfloat16, kind="ExternalInput")
    nbias = nc.dram_tensor("nbias", [GB, 1], dt.float32, kind="ExternalInput")
    ident = nc.dram_tensor("ident", [128, 128], dt.bfloat16, kind="ExternalInput")
    alpha_out = nc.dram_tensor("alpha", [BL, N], dt.float32, kind="ExternalOutput")
    ctx_out = nc.dram_tensor("context", [BL, D], dt.float32, kind="ExternalOutput")

    with tile.TileContext(nc) as tc:
        with (
            tc.tile_pool(name="const", bufs=1) as const,
            tc.tile_pool(name="encT_p", bufs=3) as encT_pool,
            tc.tile_pool(name="encN_p", bufs=4) as encN_pool,
            tc.tile_pool(name="energy_p", bufs=10) as e_pool,
            tc.tile_pool(name="row_p", bufs=2) as row_pool,
            tc.tile_pool(name="smx", bufs=2) as smx,
            tc.tile_pool(name="pe_ps", bufs=3, space="PSUM") as pe_ps,
            tc.tile_pool(name="s_ps", bufs=2, space="PSUM") as s_ps,
            tc.tile_pool(name="tp_ps", bufs=2, space="PSUM") as tp_ps,
            tc.tile_pool(name="ctx_ps", bufs=1, space="PSUM") as ctx_ps,
        ):
            w_sb = const.tile([128, 4, A], dt.bfloat16, name="w_sb")
            v_sb = const.tile([128, 4, 32], dt.bfloat16, name="v_sb")
            ph_sb = const.tile([128, 4, BL], dt.float32, name="ph_sb")
            id_sb = const.tile([128, 128], dt.bfloat16, name="id_sb")
            nb_sb = const.tile([GB, 1], dt.float32, name="nb_sb")

            encN_sb = {}
            scores_tiles = {}
            group_state = {}

            def emit_proj(b, scores_g):
                """proj_e + tanh + packed v-dot for batch b."""
                tT = encT_pool.tile([128, 4, N], dt.bfloat16, name="encT_sb")
                if b == 0:
                    # first matmuls gate on w's j=0 slice + encT chunk 0
                    nc.sync.dma_start(w_sb[:, :, 0:128], wT[:, :, 0:128])
                    nc.sync.dma_start(v_sb[:], vT[:])
                    nc.sync.dma_start(ph_sb[:], phT[:])
                    for dc in range(4):
                        nc.sync.dma_start(tT[:, dc, 0:512], encT[b][:, dc, 0:512])
                    for dc in range(4):
                        nc.sync.dma_start(
                            tT[:, dc, 512:1024], encT[b][:, dc, 512:1024]
                        )
                    nc.sync.dma_start(w_sb[:, :, 128:512], wT[:, :, 128:512])
                    for dc in range(4):
                        nc.sync.dma_start(
                            tT[:, dc, 1024:2048], encT[b][:, dc, 1024:2048]
                        )
                    nc.sync.dma_start(id_sb[:], ident[:])
                    nc.sync.dma_start(nb_sb[:], nbias[:])
                else:
                    nc.sync.dma_start(tT[:], encT[b])
                tN = encN_pool.tile([128, 16, D], dt.bfloat16, name="encN_sb")
                nc.sync.dma_start(tN[:], encN[b])
                encN_sb[b] = tN
                mrow = row_pool.tile([128, 512], dt.bfloat16, name="mrow")
                nc.sync.dma_start(mrow[:], mneg[b])
                scores_b = row_pool.tile([128, 512], dt.float32, name="scores_b")
                # scores psum: row 32*nt holds scores[nt*512:(nt+1)*512]
                s_tile = s_ps.tile([128, 512], dt.float32, name="s_psum")

                def quad(j, ens):
                    # M=32 replicated v columns fill the whole 32-row col
                    # group (same cycle count — the moving operand dominates)
                    # so every psum partition holds valid data.
                    for nt in range(4):
                        nc.tensor.matmul(
                            s_tile[32 * nt:32 * nt + 32, :],
                            v_sb[:, j, :],
                            ens[nt][:],
                            start=(j == 0),
                            stop=(j == 3),
                            tile_position=(0, 32 * nt),
                            skip_group_check=True,
                        )

                prev = None
                for j in range(4):
                    cur = []
                    for nt in range(4):
                        pe = pe_ps.tile([128, 512], dt.float32, name="pe_psum")
                        for dc in range(4):
                            nc.tensor.matmul(
                                pe[:],
                                w_sb[:, dc, j * 128:(j + 1) * 128],
                                tT[:, dc, nt * 512:(nt + 1) * 512],
                                start=(dc == 0),
                                stop=(dc == 3),
                            )
                        en = e_pool.tile([128, 512], dt.bfloat16, name="energy")
                        nc.scalar.activation(
                            en[:], pe[:], AF.Tanh, bias=ph_sb[:, j, b:b + 1]
                        )
                        cur.append(en)
                    # defer quad(j-1) so its ACT inputs are long done (no
                    # PE stall waiting on the scalar engine)
                    if prev is not None:
                        quad(j - 1, prev)
                    prev = cur
                quad(3, prev)
                # wide psum -> sbuf copies with the mask fold; rows 0-63
                # (score chunks 0-1) first so the exp half-0 gate clears early
                bl = b % GB
                nc.vector.tensor_add(
                    scores_b[0:64, :], s_tile[0:64, :], mrow[0:64, :]
                )
                nc.vector.tensor_add(
                    scores_b[64:128, :], s_tile[64:128, :], mrow[64:128, :]
                )
                # DVE/ACT cannot write partition offsets that are not
                # 32-aligned; scatter the rows into the group tile via DMA.
                for nt in range(4):
                    nc.sync.dma_start(
                        scores_g[bl:bl + 1, nt * 512:(nt + 1) * 512],
                        scores_b[32 * nt:32 * nt + 1, :],
                    )

            def emit_softmax(g):
                """exp(+sum) / bf16 cast, split into two N-halves so the
                context transposes can start after the first half.

                bias is the constant -sum(|v|): a sound softmax shift (all
                scores are below it), so no max reduction is needed."""
                scores_g = scores_tiles[g]
                expg = smx.tile([GB, N], dt.float32, name="expg")
                den0 = smx.tile([GB, 1], dt.float32, name="den0")
                den1 = smx.tile([GB, 1], dt.float32, name="den1")
                exp16 = smx.tile([GB, N], dt.bfloat16, name="exp16")
                for h, den_h in ((0, den0), (1, den1)):
                    sl = slice(h * (N // 2), (h + 1) * (N // 2))
                    nc.scalar.activation(
                        expg[:, sl], scores_g[:, sl], AF.Exp,
                        bias=nb_sb[:, 0:1], accum_out=den_h[:, 0:1],
                    )
                    nc.vector.tensor_copy(exp16[:, sl], expg[:, sl])
                den = smx.tile([GB, 1], dt.float32, name="den")
                nc.vector.tensor_add(den[:], den0[:], den1[:])
                group_state[g] = (expg, den, exp16)

            def emit_context(g):
                expg, den, exp16 = group_state[g]
                # alpha output (off the critical path): exp * (1/den)
                rden4 = smx.tile([GB, 1], dt.float32, name="rden4")
                nc.vector.reciprocal(rden4[:], den[:])
                alphag = smx.tile([GB, N], dt.float32, name="alphag")
                # ACT is idle at group tails; keep DVE free for the context
                # scale-copies
                nc.scalar.activation(
                    alphag[:], expg[:], AF.Copy, scale=rden4[:, 0:1]
                )
                nc.sync.dma_start(alpha_out[g * GB:(g + 1) * GB], alphag[:])
                # 1/den laid along free dim for the per-row context scale
                den_row = smx.tile([1, GB], dt.float32, name="den_row")
                nc.sync.dma_start(den_row[0:1, :], den[:, 0:1])
                rden_row = smx.tile([1, GB], dt.float32, name="rden_row")
                nc.vector.reciprocal(rden_row[:], den_row[:])
                # alpha columns via PE transpose + context quads, interleaved
                # per N-half so the PE starts as soon as half 0 is cast
                alphaT = smx.tile([128, 16, GB], dt.bfloat16, name="alphaT")
                ctx_tile = ctx_ps.tile([128, D], dt.float32, name="ctx_psum")
                for h in range(2):
                    for k in range(h * 8, (h + 1) * 8):
                        tp = tp_ps.tile([128, GB], dt.bfloat16, name="tp_psum")
                        nc.tensor.transpose(
                            tp[:], exp16[:, k * 128:(k + 1) * 128],
                            id_sb[0:GB, 0:GB]
                        )
                        nc.vector.tensor_copy(alphaT[:, k, :], tp[:])
                    for k in range(h * 8, (h + 1) * 8):
                        for bl in range(GB):
                            nc.tensor.matmul(
                                ctx_tile[32 * bl:32 * bl + 1, :],
                                alphaT[:, k, bl:bl + 1],
                                encN_sb[g * GB + bl][:, k, :],
                                start=(k == 0),
                                stop=(k == 15),
                                tile_position=(0, 32 * bl),
                                skip_group_check=True,
                            )
                for bl in range(GB):
                    b = g * GB + bl
                    ctx_b = row_pool.tile([1, D], dt.float32, name="ctx_b")
                    if bl % 2 == 0:
                        nc.vector.tensor_scalar(
                            ctx_b[:],
                            ctx_tile[32 * bl:32 * bl + 1, :],
                            rden_row[0:1, bl:bl + 1],
                            None,
                            op0=mybir.AluOpType.mult,
                        )
                    else:
                        # odd rows on the (idle) scalar engine, in parallel
                        nc.scalar.activation(
                            ctx_b[:], ctx_tile[32 * bl:32 * bl + 1, :],
                            AF.Copy, scale=rden_row[0:1, bl:bl + 1],
                        )
                    nc.sync.dma_start(ctx_out[b:b + 1], ctx_b[:])

            # Emission order keeps the PE stream dense: group-0 softmax
            # (DVE/ACT) is emitted before batch 4's matmuls; group-0
            # transpose+context matmuls land after batch 4 so the PE never
            # waits on the softmax chain.
            for g in range(NGROUPS):
                scores_tiles[g] = smx.tile([GB, N], dt.float32, name="scores_g")
            emit_proj(0, scores_tiles[0])
            emit_proj(1, scores_tiles[0])
            emit_proj(2, scores_tiles[0])
            emit_proj(3, scores_tiles[0])
            emit_softmax(0)
            emit_proj(4, scores_tiles[1])
            emit_context(0)
            emit_proj(5, scores_tiles[1])
            emit_proj(6, scores_tiles[1])
            emit_proj(7, scores_tiles[1])
            emit_softmax(1)
            emit_context(1)

    nc.compile()
    return nc


def _get_prog():
    global _PROG
    if _PROG is None:
        _PROG = _build()
    return _PROG


def _prep_in_maps(h, enc_mem, enc_mask, W_h, W_e, v):
    h = np.asarray(h, dtype=np.float32)
    enc_mem = np.asarray(enc_mem, dtype=np.float32)
    enc_mask = np.asarray(enc_mask)
    W_h = np.asarray(W_h, dtype=np.float32)
    W_e = np.asarray(W_e, dtype=np.float32)
    v = np.asarray(v, dtype=np.float32)

    enc16 = enc_mem.astype(BF16)
    # encT[b,p,c,n] = enc[b,n,c*128+p]
    encT_all = np.ascontiguousarray(
        enc16.reshape(B, N, 4, 128).transpose(0, 3, 2, 1)
    )
    # encN[b,p,t,d] = enc[b,t*128+p,d]
    encN_all = np.ascontiguousarray(
        enc16.reshape(B, 16, 128, D).transpose(0, 2, 1, 3)
    )
    proj_h = (h @ W_h.T).astype(np.float32)  # (B, A)
    phT_all = np.ascontiguousarray(
        proj_h.T.reshape(4, 128, B).transpose(1, 0, 2)
    )  # (128, 4, B)
    wT_host = np.ascontiguousarray(
        W_e.T.reshape(4, 128, A).transpose(1, 0, 2)
    ).astype(BF16)
    vT_host = np.ascontiguousarray(
        np.repeat(v.astype(BF16).reshape(4, 128).T[:, :, None], 32, axis=2)
    )
    # mask rows spread to partitions {0,32,64,96}: mneg[b, 32*nt, f]
    # masks scores position nt*512+f (matches the scores psum layout)
    mneg_flat = ((enc_mask.astype(np.float32) - 1.0) * 1.0e9).astype(BF16)
    mneg_all = np.zeros((B, 128, 512), dtype=BF16)
    mneg_all[:, 0:128:32, :] = mneg_flat.reshape(B, 4, 512)
    ident = np.eye(128, dtype=BF16)
    nbias_host = np.full((GB, 1), -float(np.abs(v).sum()), dtype=np.float32)

    in_maps = []
    for c in range(NCORES):
        sl = slice(c * BL, (c + 1) * BL)
        in_maps.append({
            "encT": encT_all[sl],
            "encN": encN_all[sl],
            "wT": wT_host,
            "vT": vT_host,
            "phT": np.ascontiguousarray(phT_all[:, :, sl]),
            "mneg": np.ascontiguousarray(mneg_all[sl]),
            "nbias": nbias_host,
            "ident": ident,
        })
    return in_maps


def _gather(results):
    context = np.concatenate(
        [results[c]["context"] for c in range(NCORES)], axis=0
    ).astype(np.float32)
    alpha = np.concatenate(
        [results[c]["alpha"] for c in range(NCORES)], axis=0
    ).astype(np.float32)
    return context, alpha


def _reset_devices():
    """Recover a wedged NRT exec unit (best-effort)."""
    try:
        import ctypes
        import jax
        jax.devices()
        lib = ctypes.CDLL("/opt/axon/libaxon_pjrt.so")
        lib.axon_reset.restype = ctypes.c_int64
        lib.axon_reset()
    except Exception:
        pass


def run(inputs, trace=False):
    """Run on the 8 NeuronCores; returns ((context, alpha), exec_time_ns)."""
    from concourse.bass_utils import run_bass_kernel_spmd

    nc = _get_prog()
    in_maps = _prep_in_maps(**inputs)
    res = run_bass_kernel_spmd(nc, in_maps, list(range(NCORES)), trace=trace)
    return _gather(res.results), res.exec_time_ns


def kernel(h, enc_mem, enc_mask, W_h, W_e, v):
    inputs = dict(h=h, enc_mem=enc_mem, enc_mask=enc_mask, W_h=W_h, W_e=W_e, v=v)
    try:
        (context, alpha), _ = run(inputs)
    except Exception:
        _reset_devices()
        (context, alpha), _ = run(inputs)
    return context, alpha


# revision 74
# speedup vs baseline: 1.0201x; 1.0201x over previous
"""Bahdanau attention Trainium2 kernel.

Problem sizes (hardcoded): B=64, N=2048, H=D=A=512, 8 NeuronCores,
data-parallel over B (8 batches per core, replicated weights).

Math per batch b:
    proj_h = h @ W_h.T                   (host: 0.05% of FLOPs)
    proj_e = enc_mem[b] @ W_e.T          (PE, bf16 inputs, fp32 accum)
    energy = tanh(proj_h + proj_e)       (ACT, bias fused)
    scores = energy @ v                  (PE, col-group packed 4x)
    scores += (mask-1)*1e9               (DVE, fused into psum->sbuf copy)
    alpha  = softmax(scores)             (constant-shift exp: bias=-sum|v|)
    context = (exp @ enc_mem[b]) / den   (PE col-group packed 4x)

Device layouts (per core, host-marshalled):
    encT  [8,128,4,2048] bf16 : encT[b,p,c,n] = enc[b,n,c*128+p]   (d-major)
    encN  [8,128,16,512] bf16 : encN[b,p,t,d] = enc[b,t*128+p,d]   (n-major)
    wT    [128,4,512]    bf16 : wT[p,c,a]     = W_e[a,c*128+p]
    vT    [128,4]        bf16 : vT[p,j]       = v[j*128+p]
    phT   [128,4,8]      f32  : phT[p,c,b]    = proj_h[b,c*128+p]
    mneg  [8,128,512]    bf16 : (mask-1)*1e9, rows spread to {0,32,64,96}
    nbias [4,1]          f32  : -sum(|v|) (constant softmax shift)
    ident [128,128]      bf16 : identity for PE transpose
Outputs: alpha [8,2048] f32, context [8,512] f32.
"""

import numpy as np
import ml_dtypes

B, N, D, A = 64, 2048, 512, 512
NCORES = 8
BL = B // NCORES          # 8 batches per core
GB = 4                    # batches per softmax group
NGROUPS = BL // GB        # 2 groups
BF16 = ml_dtypes.bfloat16

_PROG = None


def _build():
    import concourse.bass as bass
    import concourse.mybir as mybir
    import concourse.tile as tile
    from concourse import bacc

    dt = mybir.dt
    AF = mybir.ActivationFunctionType
    AX = mybir.AxisListType

    nc = bacc.Bacc("TRN2", target_bir_lowering=False, debug=False)

    encT = nc.dram_tensor("encT", [BL, 128, 4, N], dt.bfloat16, kind="ExternalInput")
    encN = nc.dram_tensor("encN", [BL, 128, 16, D], dt.bfloat16, kind="ExternalInput")
    wT = nc.dram_tensor("wT", [128, 4, A], dt.bfloat16, kind="ExternalInput")
    vT = nc.dram_tensor("vT", [128, 4, 32], dt.bfloat16, kind="ExternalInput")
    phT = nc.dram_tensor("phT", [128, 4, BL], dt.float32, kind="ExternalInput")
    mneg = nc.dram_tensor("mneg", [BL, 128, 512], dt.bfloat16, kind="ExternalInput")
    nbias = nc.dram_tensor("nbias", [GB, 1], dt.float32, kind="ExternalInput")
    ident = nc.dram_tensor("ident", [128, 128], dt.bfloat16, kind="ExternalInput")
    alpha_out = nc.dram_tensor("alpha", [BL, N], dt.float32, kind="ExternalOutput")
    ctx_out = nc.dram_tensor("context", [BL, D], dt.float32, kind="ExternalOutput")

    with tile.TileContext(nc) as tc:
        with (
            tc.tile_pool(name="const", bufs=1) as const,
            tc.tile_pool(name="encT_p", bufs=3) as encT_pool,
            tc.tile_pool(name="encN_p", bufs=4) as encN_pool,
            tc.tile_pool(name="energy_p", bufs=10) as e_pool,
            tc.tile_pool(name="row_p", bufs=2) as row_pool,
            tc.tile_pool(name="smx", bufs=2) as smx,
            tc.tile_pool(name="pe_ps", bufs=3, space="PSUM") as pe_ps,
            tc.tile_pool(name="s_ps", bufs=2, space="PSUM") as s_ps,
            tc.tile_pool(name="tp_ps", bufs=2, space="PSUM") as tp_ps,
            tc.tile_pool(name="ctx_ps", bufs=1, space="PSUM") as ctx_ps,
        ):
            w_sb = const.tile([128, 4, A], dt.bfloat16, name="w_sb")
            v_sb = const.tile([128, 4, 32], dt.bfloat16, name="v_sb")
            ph_sb = const.tile([128, 4, BL], dt.float32, name="ph_sb")
            id_sb = const.tile([128, 128], dt.bfloat16, name="id_sb")
            nb_sb = const.tile([GB, 1], dt.float32, name="nb_sb")

            encN_sb = {}
            scores_tiles = {}
            group_state = {}

            def emit_proj(b, scores_g):
                """proj_e + tanh + packed v-dot for batch b."""
                tT = encT_pool.tile([128, 4, N], dt.bfloat16, name="encT_sb")
                if b == 0:
                    # first matmuls gate on w's j=0 slice + encT chunk 0
                    nc.sync.dma_start(w_sb[:, :, 0:128], wT[:, :, 0:128])
                    nc.sync.dma_start(v_sb[:], vT[:])
                    nc.sync.dma_start(ph_sb[:], phT[:])
                    for dc in range(4):
                        nc.sync.dma_start(tT[:, dc, 0:512], encT[b][:, dc, 0:512])
                    for dc in range(4):
                        nc.sync.dma_start(
                            tT[:, dc, 512:1024], encT[b][:, dc, 512:1024]
                        )
                    nc.sync.dma_start(w_sb[:, :, 128:512], wT[:, :, 128:512])
                    for dc in range(4):
                        nc.sync.dma_start(
                            tT[:, dc, 1024:2048], encT[b][:, dc, 1024:2048]
                        )
                    nc.sync.dma_start(id_sb[:], ident[:])
                    nc.sync.dma_start(nb_sb[:], nbias[:])
                else:
                    nc.sync.dma_start(tT[:], encT[b])
                tN = encN_pool.tile([128, 16, D], dt.bfloat16, name="encN_sb")
                nc.sync.dma_start(tN[:], encN[b])
                encN_sb[b] = tN
                mrow = row_pool.tile([128, 512], dt.bfloat16, name="mrow")
                nc.sync.dma_start(mrow[:], mneg[b])
                scores_b = row_pool.tile([128, 512], dt.float32, name="scores_b")
                # scores psum: row 32*nt holds scores[nt*512:(nt+1)*512]
                s_tile = s_ps.tile([128, 512], dt.float32, name="s_psum")

                def quad(j, ens):
                    # M=32 replicated v columns fill the whole 32-row col
                    # group (same cycle count — the moving operand dominates)
                    # so every psum partition holds valid data.
                    for nt in range(4):
                        nc.tensor.matmul(
                            s_tile[32 * nt:32 * nt + 32, :],
                            v_sb[:, j, :],
                            ens[nt][:],
                            start=(j == 0),
                            stop=(j == 3),
                            tile_position=(0, 32 * nt),
                            skip_group_check=True,
                        )

                prev = None
                for j in range(4):
                    cur = []
                    for nt in range(4):
                        pe = pe_ps.tile([128, 512], dt.float32, name="pe_psum")
                        for dc in range(4):
                            nc.tensor.matmul(
                                pe[:],
                                w_sb[:, dc, j * 128:(j + 1) * 128],
                                tT[:, dc, nt * 512:(nt + 1) * 512],
                                start=(dc == 0),
                                stop=(dc == 3),
                            )
                        en = e_pool.tile([128, 512], dt.bfloat16, name="energy")
                        nc.scalar.activation(
                            en[:], pe[:], AF.Tanh, bias=ph_sb[:, j, b:b + 1]
                        )
                        cur.append(en)
                    # defer quad(j-1) so its ACT inputs are long done (no
                    # PE stall waiting on the scalar engine)
                    if prev is not None:
                        quad(j - 1, prev)
                    prev = cur
                quad(3, prev)
                # wide psum -> sbuf copies with the mask fold; rows 0-63
                # (score chunks 0-1) first so the exp half-0 gate clears early
                bl = b % GB
                nc.vector.tensor_add(
                    scores_b[0:64, :], s_tile[0:64, :], mrow[0:64, :]
                )
                nc.vector.tensor_add(
                    scores_b[64:128, :], s_tile[64:128, :], mrow[64:128, :]
                )
                # DVE/ACT cannot write partition offsets that are not
                # 32-aligned; scatter the rows into the group tile via DMA.
                for nt in range(4):
                    nc.sync.dma_start(
                        scores_g[bl:bl + 1, nt * 512:(nt + 1) * 512],
                        scores_b[32 * nt:32 * nt + 1, :],
                    )

            def emit_softmax(g):
                """exp(+sum) / bf16 cast, split into two N-halves so the
                context transposes can start after the first half.

                bias is the constant -sum(|v|): a sound softmax shift (all
                scores are below it), so no max reduction is needed."""
                scores_g = scores_tiles[g]
                # bf16 exp straight from ACT, halves first: this is the only
                # gate for the context transposes. The fp32 exp for the alpha
                # output (+den accum) runs afterwards, hidden under the
                # transpose/context matmuls.
                exp16 = smx.tile([GB, N], dt.bfloat16, name="exp16")
                for h in range(2):
                    sl = slice(h * (N // 2), (h + 1) * (N // 2))
                    nc.scalar.activation(
                        exp16[:, sl], scores_g[:, sl], AF.Exp, bias=nb_sb[:, 0:1]
                    )
                expg = smx.tile([GB, N], dt.float32, name="expg")
                den = smx.tile([GB, 1], dt.float32, name="den")
                nc.scalar.activation(
                    expg[:], scores_g[:], AF.Exp,
                    bias=nb_sb[:, 0:1], accum_out=den[:, 0:1],
                )
                group_state[g] = (expg, den, exp16)

            def emit_context(g):
                expg, den, exp16 = group_state[g]
                # alpha output (off the critical path): exp * (1/den)
                rden4 = smx.tile([GB, 1], dt.float32, name="rden4")
                nc.vector.reciprocal(rden4[:], den[:])
                alphag = smx.tile([GB, N], dt.float32, name="alphag")
                # ACT is idle at group tails; keep DVE free for the context
                # scale-copies
                nc.scalar.activation(
                    alphag[:], expg[:], AF.Copy, scale=rden4[:, 0:1]
                )
                nc.sync.dma_start(alpha_out[g * GB:(g + 1) * GB], alphag[:])
                # 1/den laid along free dim for the per-row context scale
                den_row = smx.tile([1, GB], dt.float32, name="den_row")
                nc.sync.dma_start(den_row[0:1, :], den[:, 0:1])
                rden_row = smx.tile([1, GB], dt.float32, name="rden_row")
                nc.vector.reciprocal(rden_row[:], den_row[:])
                # alpha columns via PE transpose + context quads, interleaved
                # per N-half so the PE starts as soon as half 0 is cast
                alphaT = smx.tile([128, 16, GB], dt.bfloat16, name="alphaT")
                ctx_tile = ctx_ps.tile([128, D], dt.float32, name="ctx_psum")
                for h in range(2):
                    for k in range(h * 8, (h + 1) * 8):
                        tp = tp_ps.tile([128, GB], dt.bfloat16, name="tp_psum")
                        nc.tensor.transpose(
                            tp[:], exp16[:, k * 128:(k + 1) * 128],
                            id_sb[0:GB, 0:GB]
                        )
                        nc.vector.tensor_copy(alphaT[:, k, :], tp[:])
                    for k in range(h * 8, (h + 1) * 8):
                        for bl in range(GB):
                            nc.tensor.matmul(
                                ctx_tile[32 * bl:32 * bl + 1, :],
                                alphaT[:, k, bl:bl + 1],
                                encN_sb[g * GB + bl][:, k, :],
                                start=(k == 0),
                                stop=(k == 15),
                                tile_position=(0, 32 * bl),
                                skip_group_check=True,
                            )
                # scale rows into one [1, 4*D] tile -> single output DMA
                ctx_row = row_pool.tile([1, GB, D], dt.float32, name="ctx_row")
                for bl in range(GB):
                    if bl % 2 == 0:
                        nc.vector.tensor_scalar(
                            ctx_row[0:1, bl, :],
                            ctx_tile[32 * bl:32 * bl + 1, :],
                            rden_row[0:1, bl:bl + 1],
                            None,
                            op0=mybir.AluOpType.mult,
                        )
                    else:
                        # odd rows on the (idle) scalar engine, in parallel
                        nc.scalar.activation(
                            ctx_row[0:1, bl, :], ctx_tile[32 * bl:32 * bl + 1, :],
                            AF.Copy, scale=rden_row[0:1, bl:bl + 1],
                        )
                nc.sync.dma_start(ctx_out[g * GB:(g + 1) * GB], ctx_row[:])

            # Emission order keeps the PE stream dense: group-0 softmax
            # (DVE/ACT) is emitted before batch 4's matmuls; group-0
            # transpose+context matmuls land after batch 4 so the PE never
            # waits on the softmax chain.
            for g in range(NGROUPS):
                scores_tiles[g] = smx.tile([GB, N], dt.float32, name="scores_g")
            emit_proj(0, scores_tiles[0])
            emit_proj(1, scores_tiles[0])
            emit_proj(2, scores_tiles[0])
            emit_proj(3, scores_tiles[0])
            emit_softmax(0)
            emit_proj(4, scores_tiles[1])
            emit_context(0)
            emit_proj(5, scores_tiles[1])
            emit_proj(6, scores_tiles[1])
            emit_proj(7, scores_tiles[1])
            emit_softmax(1)
            emit_context(1)

    nc.compile()
    return nc


def _get_prog():
    global _PROG
    if _PROG is None:
        _PROG = _build()
    return _PROG


def _prep_in_maps(h, enc_mem, enc_mask, W_h, W_e, v):
    h = np.asarray(h, dtype=np.float32)
    enc_mem = np.asarray(enc_mem, dtype=np.float32)
    enc_mask = np.asarray(enc_mask)
    W_h = np.asarray(W_h, dtype=np.float32)
    W_e = np.asarray(W_e, dtype=np.float32)
    v = np.asarray(v, dtype=np.float32)

    enc16 = enc_mem.astype(BF16)
    # encT[b,p,c,n] = enc[b,n,c*128+p]
    encT_all = np.ascontiguousarray(
        enc16.reshape(B, N, 4, 128).transpose(0, 3, 2, 1)
    )
    # encN[b,p,t,d] = enc[b,t*128+p,d]
    encN_all = np.ascontiguousarray(
        enc16.reshape(B, 16, 128, D).transpose(0, 2, 1, 3)
    )
    proj_h = (h @ W_h.T).astype(np.float32)  # (B, A)
    phT_all = np.ascontiguousarray(
        proj_h.T.reshape(4, 128, B).transpose(1, 0, 2)
    )  # (128, 4, B)
    wT_host = np.ascontiguousarray(
        W_e.T.reshape(4, 128, A).transpose(1, 0, 2)
    ).astype(BF16)
    vT_host = np.ascontiguousarray(
        np.repeat(v.astype(BF16).reshape(4, 128).T[:, :, None], 32, axis=2)
    )
    # mask rows spread to partitions {0,32,64,96}: mneg[b, 32*nt, f]
    # masks scores position nt*512+f (matches the scores psum layout)
    mneg_flat = ((enc_mask.astype(np.float32) - 1.0) * 1.0e9).astype(BF16)
    mneg_all = np.zeros((B, 128, 512), dtype=BF16)
    mneg_all[:, 0:128:32, :] = mneg_flat.reshape(B, 4, 512)
    ident = np.eye(128, dtype=BF16)
    nbias_host = np.full((GB, 1), -float(np.abs(v).sum()), dtype=np.float32)

    in_maps = []
    for c in range(NCORES):
        sl = slice(c * BL, (c + 1) * BL)
        in_maps.append({
            "encT": encT_all[sl],
            "encN": encN_all[sl],
            "wT": wT_host,
            "vT": vT_host,
            "phT": np.ascontiguousarray(phT_all[:, :, sl]),
            "mneg": np.ascontiguousarray(mneg_all[sl]),
            "nbias": nbias_host,
            "ident": ident,
        })
    return in_maps


def _gather(results):
    context = np.concatenate(
        [results[c]["context"] for c in range(NCORES)], axis=0
    ).astype(np.float32)
    alpha = np.concatenate(
        [results[c]["alpha"] for c in range(NCORES)], axis=0
    ).astype(np.float32)
    return context, alpha


def _reset_devices():
    """Recover a wedged NRT exec unit (best-effort)."""
    try:
        import ctypes
        import jax
        jax.devices()
        lib = ctypes.CDLL("/opt/axon/libaxon_pjrt.so")
        lib.axon_reset.restype = ctypes.c_int64
        lib.axon_reset()
    except Exception:
        pass


def run(inputs, trace=False):
    """Run on the 8 NeuronCores; returns ((context, alpha), exec_time_ns)."""
    from concourse.bass_utils import run_bass_kernel_spmd

    nc = _get_prog()
    in_maps = _prep_in_maps(**inputs)
    res = run_bass_kernel_spmd(nc, in_maps, list(range(NCORES)), trace=trace)
    return _gather(res.results), res.exec_time_ns


def kernel(h, enc_mem, enc_mask, W_h, W_e, v):
    inputs = dict(h=h, enc_mem=enc_mem, enc_mask=enc_mask, W_h=W_h, W_e=W_e, v=v)
    try:
        (context, alpha), _ = run(inputs)
    except Exception:
        _reset_devices()
        (context, alpha), _ = run(inputs)
    return context, alpha
